# revision 1
# baseline (speedup 1.0000x reference)
"""MoE batched-experts kernel for Trainium2 (8 NeuronCores, expert-parallel).

Contract: kernel(**inputs) takes the FULL unsharded inputs
  x:              [T, D]      float32   (T=16384, D=1024)
  expert_indices: [T]         int32     (values in [0, 8))
  gate_up_weight: [E, 2F, D]  float32   (E=8, F=2048)
  down_weight:    [E, D, F]   float32
and returns the FULL output [T, D] float32:
  per token t with expert e: h = silu(x @ gu[e,:F].T) * (x @ gu[e,F:].T);
  out = h @ dw[e].T

Strategy: expert-parallel. The host routes (sorts) tokens by expert, pads
each expert's group to a common capacity C, and core e runs a dense FFN for
expert e on its token group. All operands are pre-transposed / pre-cast to
bf16 on the host so the device kernel is pure matmul + silu*mul with no
on-chip transposes:
  core e computes outT = w_d @ (silu(w_gT.T @ xT) * (w_uT.T @ xT))
with xT [D, C], producing outT [D, C] fp32; the host transposes back and
unpermutes.
"""

import numpy as np
import ml_dtypes

import concourse.bass as bass
import concourse.mybir as mybir
from concourse import bacc
from concourse.tile import TileContext
from concourse.bass import ts, ds
from concourse.bass_utils import run_bass_kernel_spmd
from contextlib import ExitStack

BF16 = ml_dtypes.bfloat16

D = 1024      # d_model
F = 2048      # d_ff
F2 = 2 * F    # gate+up
E = 8         # experts == cores
KD = D // 128   # 8  k-tiles over d_model
KF = F // 128   # 16 k-tiles over d_ff
MD = D // 128   # 8  m-tiles over d_model (output)
NT = 512        # token chunk (one PSUM bank at fp32)

_nc_cache = {}


def build_nc(C):
    """Build the per-core dense-FFN Bass program for token capacity C."""
    nc = bacc.Bacc("TRN2", target_bir_lowering=False, debug=False, num_devices=E)
    dt = mybir.dt
    xT = nc.dram_tensor("xT", [D, C], dt.bfloat16, kind="ExternalInput")
    wgu = nc.dram_tensor("wguT", [D, F2], dt.bfloat16, kind="ExternalInput")
    wd = nc.dram_tensor("wdT", [F, D], dt.bfloat16, kind="ExternalInput")
    outT = nc.dram_tensor("outT", [D, C], dt.float32, kind="ExternalOutput")

    with TileContext(nc) as tc, ExitStack() as ctx:
        wpool = ctx.enter_context(tc.tile_pool(name="weights", bufs=1))
        wgu_sb = wpool.tile([128, KD, F2], dt.bfloat16, tag="wgu")
        nc.sync.dma_start(wgu_sb[:], wgu.rearrange("(k p) f -> p k f", p=128))
        wd_sb = wpool.tile([128, KF, D], dt.bfloat16, tag="wd")
        nc.sync.dma_start(wd_sb[:], wd.rearrange("(k p) f -> p k f", p=128))

        xpool = ctx.enter_context(tc.tile_pool(name="x", bufs=2))
        hpool = ctx.enter_context(tc.tile_pool(name="h", bufs=2))
        spool = ctx.enter_context(tc.tile_pool(name="silu", bufs=4))
        opool = ctx.enter_context(tc.tile_pool(name="o", bufs=4))
        pg = ctx.enter_context(tc.tile_pool(name="pg", bufs=2, space="PSUM"))
        pu = ctx.enter_context(tc.tile_pool(name="pu", bufs=2, space="PSUM"))
        po = ctx.enter_context(tc.tile_pool(name="po", bufs=2, space="PSUM"))

        xT_r = xT.rearrange("(k p) t -> p k t", p=128)
        outT_r = outT.rearrange("(m p) t -> p m t", p=128)

        for n0 in range(0, C, NT):
            nt = min(NT, C - n0)
            x_sb = xpool.tile([128, KD, NT], dt.bfloat16)
            nc.sync.dma_start(x_sb[:, :, :nt], xT_r[:, :, n0:n0 + nt])
            h_sb = hpool.tile([128, KF, NT], dt.bfloat16)
            for mp in range(KF):
                psg = pg.tile([128, NT], dt.float32)
                for k in range(KD):
                    nc.tensor.matmul(
                        psg[:, :nt], lhsT=wgu_sb[:, k, ts(mp, 128)],
                        rhs=x_sb[:, k, :nt], start=(k == 0), stop=(k == KD - 1))
                psu = pu.tile([128, NT], dt.float32)
                for k in range(KD):
                    nc.tensor.matmul(
                        psu[:, :nt], lhsT=wgu_sb[:, k, ds(F + mp * 128, 128)],
                        rhs=x_sb[:, k, :nt], start=(k == 0), stop=(k == KD - 1))
                sil = spool.tile([128, NT], dt.bfloat16)
                nc.scalar.activation(sil[:, :nt], psg[:, :nt],
                                     mybir.ActivationFunctionType.Silu)
                nc.vector.tensor_mul(h_sb[:, mp, :nt], sil[:, :nt], psu[:, :nt])
            for m in range(MD):
                pso = po.tile([128, NT], dt.float32)
                for k in range(KF):
                    nc.tensor.matmul(
                        pso[:, :nt], lhsT=wd_sb[:, k, ts(m, 128)],
                        rhs=h_sb[:, k, :nt], start=(k == 0), stop=(k == KF - 1))
                o_sb = opool.tile([128, NT], dt.float32)
                nc.vector.tensor_copy(o_sb[:, :nt], pso[:, :nt])
                nc.sync.dma_start(outT_r[:, m, n0:n0 + nt], o_sb[:, :nt])
    nc.finalize()
    return nc


def get_nc(C):
    if C not in _nc_cache:
        _nc_cache[C] = build_nc(C)
    return _nc_cache[C]


def route(x, expert_indices):
    """Sort tokens by expert; return (order, counts, capacity C)."""
    idx = np.asarray(expert_indices)
    order = np.argsort(idx, kind="stable")
    counts = np.bincount(idx, minlength=E).astype(np.int64)
    C = max(NT, int(-(-counts.max() // 128) * 128))
    return order, counts, C


def make_in_maps(x, expert_indices, gate_up_weight, down_weight):
    order, counts, C = route(x, expert_indices)
    x_sorted = np.asarray(x, dtype=np.float32)[order]
    offs = np.zeros(E + 1, dtype=np.int64)
    np.cumsum(counts, out=offs[1:])
    wguT = np.ascontiguousarray(
        np.transpose(np.asarray(gate_up_weight), (0, 2, 1))).astype(BF16)
    wdT = np.ascontiguousarray(
        np.transpose(np.asarray(down_weight), (0, 2, 1))).astype(BF16)
    in_maps = []
    for e in range(E):
        xe = np.zeros((C, D), dtype=np.float32)
        xe[: counts[e]] = x_sorted[offs[e]: offs[e + 1]]
        in_maps.append({
            "xT": np.ascontiguousarray(xe.T).astype(BF16),
            "wguT": wguT[e],
            "wdT": wdT[e],
        })
    return in_maps, order, counts, C


def assemble_output(results, order, counts):
    T = int(counts.sum())
    out = np.empty((T, D), dtype=np.float32)
    offs = np.zeros(E + 1, dtype=np.int64)
    np.cumsum(counts, out=offs[1:])
    sorted_out = np.empty((T, D), dtype=np.float32)
    for e in range(E):
        sorted_out[offs[e]: offs[e + 1]] = results[e]["outT"].T[: counts[e]]
    out[order] = sorted_out
    return out


def kernel(x, expert_indices, gate_up_weight, down_weight):
    in_maps, order, counts, C = make_in_maps(
        x, expert_indices, gate_up_weight, down_weight)
    nc = get_nc(C)
    res = run_bass_kernel_spmd(nc, in_maps, core_ids=list(range(E)))
    return assemble_output(res.results, order, counts)


# revision 9
# speedup vs baseline: 7.5393x; 7.5393x over previous
"""MoE batched-experts kernel for Trainium2 (8 NeuronCores, expert-parallel).

Contract: kernel(**inputs) takes the FULL unsharded inputs
  x:              [T, D]      float32   (T=16384, D=1024)
  expert_indices: [T]         int32     (values in [0, 8))
  gate_up_weight: [E, 2F, D]  float32   (E=8, F=2048)
  down_weight:    [E, D, F]   float32
and returns the FULL output [T, D] float32:
  per token t with expert e: h = silu(x @ gu[e,:F].T) * (x @ gu[e,F:].T);
  out = h @ dw[e].T

Strategy: expert-parallel. The host routes (sorts) tokens by expert, pads
each expert's group to a common capacity C, and core e runs a dense FFN for
expert e on its token group. All operands are pre-transposed / pre-cast to
bf16 on the host so the device kernel is pure matmul + silu*mul with no
on-chip transposes:
  core e computes outT = w_d @ (silu(w_gT.T @ xT) * (w_uT.T @ xT))
with xT [D, C], producing outT [D, C] fp32; the host transposes back and
unpermutes.
"""

import numpy as np
import ml_dtypes

import concourse.bass as bass
import concourse.mybir as mybir
from concourse import bacc
from concourse.tile import TileContext
from concourse.bass import ts, ds
from concourse.bass_utils import run_bass_kernel_spmd
from contextlib import ExitStack

BF16 = ml_dtypes.bfloat16

D = 1024      # d_model
F = 2048      # d_ff
F2 = 2 * F    # gate+up
E = 8         # experts == cores
KD = D // 128   # 8  k-tiles over d_model
KF = F // 128   # 16 k-tiles over d_ff
MD = D // 128   # 8  m-tiles over d_model (output)
NT = 512        # token chunk (one PSUM bank at fp32)

_nc_cache = {}


def build_nc(C, repeats=1, hw_loop=0):
    """Build the per-core dense-FFN Bass program for token capacity C.

    repeats>1 re-emits the whole compute body (unrolled); hw_loop>0 wraps the
    body in a hardware For_i loop. Both are timing aids: slope of time vs
    repetition count isolates true exec time from dispatch overhead."""
    nc = bacc.Bacc("TRN2", target_bir_lowering=False, debug=False, num_devices=E)
    dt = mybir.dt
    xT = nc.dram_tensor("xT", [D, C], dt.bfloat16, kind="ExternalInput")
    wgu = nc.dram_tensor("wguT", [D, F2], dt.bfloat16, kind="ExternalInput")
    wd = nc.dram_tensor("wdT", [F, D], dt.bfloat16, kind="ExternalInput")
    outT = nc.dram_tensor("outT", [D, C], dt.float32, kind="ExternalOutput")

    with TileContext(nc) as tc, ExitStack() as ctx:
        wpool = ctx.enter_context(tc.tile_pool(name="weights", bufs=1))
        wgu_sb = wpool.tile([128, KD, F2], dt.bfloat16, tag="wgu")
        nc.sync.dma_start(wgu_sb[:], wgu.rearrange("(k p) f -> p k f", p=128))
        wd_sb = wpool.tile([128, KF, D], dt.bfloat16, tag="wd")
        nc.sync.dma_start(wd_sb[:], wd.rearrange("(k p) f -> p k f", p=128))

        xpool = ctx.enter_context(tc.tile_pool(name="x", bufs=2))
        hpool = ctx.enter_context(tc.tile_pool(name="h", bufs=2))
        spool = ctx.enter_context(tc.tile_pool(name="silu", bufs=4))
        opool = ctx.enter_context(tc.tile_pool(name="o", bufs=4))
        pg = ctx.enter_context(tc.tile_pool(name="pg", bufs=2, space="PSUM"))
        pu = ctx.enter_context(tc.tile_pool(name="pu", bufs=2, space="PSUM"))
        po = ctx.enter_context(tc.tile_pool(name="po", bufs=2, space="PSUM"))

        xT_r = xT.rearrange("(k p) t -> p k t", p=128)
        outT_r = outT.rearrange("(m p) t -> p m t", p=128)

        def body():
            for n0 in [i for _ in range(repeats) for i in range(0, C, NT)]:
                nt = min(NT, C - n0)
                x_sb = xpool.tile([128, KD, NT], dt.bfloat16, tag="x")
                nc.sync.dma_start(x_sb[:, :, :nt], xT_r[:, :, n0:n0 + nt])
                h_sb = hpool.tile([128, KF, NT], dt.bfloat16, tag="h")
                for mp in range(KF):
                    psg = pg.tile([128, NT], dt.float32, tag="pg")
                    for k in range(KD):
                        nc.tensor.matmul(
                            psg[:, :nt], lhsT=wgu_sb[:, k, ts(mp, 128)],
                            rhs=x_sb[:, k, :nt], start=(k == 0), stop=(k == KD - 1))
                    psu = pu.tile([128, NT], dt.float32, tag="pu")
                    for k in range(KD):
                        nc.tensor.matmul(
                            psu[:, :nt], lhsT=wgu_sb[:, k, ds(F + mp * 128, 128)],
                            rhs=x_sb[:, k, :nt], start=(k == 0), stop=(k == KD - 1))
                    sil = spool.tile([128, NT], dt.bfloat16, tag="sil")
                    nc.scalar.activation(sil[:, :nt], psg[:, :nt],
                                         mybir.ActivationFunctionType.Silu)
                    nc.vector.tensor_mul(h_sb[:, mp, :nt], sil[:, :nt], psu[:, :nt])
                for m in range(MD):
                    pso = po.tile([128, NT], dt.float32, tag="po")
                    for k in range(KF):
                        nc.tensor.matmul(
                            pso[:, :nt], lhsT=wd_sb[:, k, ts(m, 128)],
                            rhs=h_sb[:, k, :nt], start=(k == 0), stop=(k == KF - 1))
                    o_sb = opool.tile([128, NT], dt.float32, tag="o")
                    nc.vector.tensor_copy(o_sb[:, :nt], pso[:, :nt])
                    nc.sync.dma_start(outT_r[:, m, n0:n0 + nt], o_sb[:, :nt])

        if hw_loop:
            with tc.For_i(0, hw_loop, 1):
                body()
        else:
            body()
    nc.finalize()
    return nc


def build_nc_wide(C, hw_loop=0):
    """Variant: 1024-token compute chunks with [128,1024] PSUM tiles.

    - halves ACT/DVE eviction instruction count (wide silu/mul)
    - consecutive matmuls share the same lhsT (LDW dedup opportunity)
    - PSUM banks: pg 2x2 + pu 1x2 + po 2x1 = 8
    """
    nc = bacc.Bacc("TRN2", target_bir_lowering=False, debug=False, num_devices=E)
    dt = mybir.dt
    NW = 1024
    xT = nc.dram_tensor("xT", [D, C], dt.bfloat16, kind="ExternalInput")
    wgu = nc.dram_tensor("wguT", [D, F2], dt.bfloat16, kind="ExternalInput")
    wd = nc.dram_tensor("wdT", [F, D], dt.bfloat16, kind="ExternalInput")
    outT = nc.dram_tensor("outT", [D, C], dt.float32, kind="ExternalOutput")

    with TileContext(nc) as tc, ExitStack() as ctx:
        wpool = ctx.enter_context(tc.tile_pool(name="weights", bufs=1))
        wgu_sb = wpool.tile([128, KD, F2], dt.bfloat16, tag="wgu")
        nc.sync.dma_start(wgu_sb[:], wgu.rearrange("(k p) f -> p k f", p=128))
        wd_sb = wpool.tile([128, KF, D], dt.bfloat16, tag="wd")
        nc.sync.dma_start(wd_sb[:], wd.rearrange("(k p) f -> p k f", p=128))

        xpool = ctx.enter_context(tc.tile_pool(name="x", bufs=1))
        hpool = ctx.enter_context(tc.tile_pool(name="h", bufs=3))
        spool = ctx.enter_context(tc.tile_pool(name="silu", bufs=3))
        opool = ctx.enter_context(tc.tile_pool(name="o", bufs=4))
        pg = ctx.enter_context(tc.tile_pool(name="pg", bufs=2, space="PSUM"))
        pu = ctx.enter_context(tc.tile_pool(name="pu", bufs=1, space="PSUM"))
        po = ctx.enter_context(tc.tile_pool(name="po", bufs=2, space="PSUM"))

        xT_r = xT.rearrange("(k p) t -> p k t", p=128)
        outT_r = outT.rearrange("(m p) t -> p m t", p=128)

        def do_chunk(n0, nw):
            # nw tokens starting at n0; nw in {1024, C % 1024}
            nh = (nw + NT - 1) // NT  # h sub-chunks of <=512
            x_sb = xpool.tile([128, KD, NW], dt.bfloat16, tag="x")
            nc.sync.dma_start(x_sb[:, :, :nw], xT_r[:, :, n0:n0 + nw])
            h_sbs = [hpool.tile([128, KF, NT], dt.bfloat16, tag="h",
                                name=f"h_{n0}_{s}")
                     for s in range(nh)]
            for mp in range(KF):
                psg = pg.tile([128, NW], dt.float32, tag="pg")
                for k in range(KD):
                    for s in range(nh):
                        w = min(NT, nw - s * NT)
                        nc.tensor.matmul(
                            psg[:, s * NT:s * NT + w],
                            lhsT=wgu_sb[:, k, ts(mp, 128)],
                            rhs=x_sb[:, k, s * NT:s * NT + w],
                            start=(k == 0), stop=(k == KD - 1))
                psu = pu.tile([128, NW], dt.float32, tag="pu")
                for k in range(KD):
                    for s in range(nh):
                        w = min(NT, nw - s * NT)
                        nc.tensor.matmul(
                            psu[:, s * NT:s * NT + w],
                            lhsT=wgu_sb[:, k, ds(F + mp * 128, 128)],
                            rhs=x_sb[:, k, s * NT:s * NT + w],
                            start=(k == 0), stop=(k == KD - 1))
                sil = spool.tile([128, NW], dt.bfloat16, tag="sil")
                nc.scalar.activation(sil[:, :nw], psg[:, :nw],
                                     mybir.ActivationFunctionType.Silu)
                for s in range(nh):
                    w = min(NT, nw - s * NT)
                    nc.vector.tensor_mul(h_sbs[s][:, mp, :w],
                                         sil[:, s * NT:s * NT + w],
                                         psu[:, s * NT:s * NT + w])
            for m in range(MD):
                for s in range(nh):
                    w = min(NT, nw - s * NT)
                    pso = po.tile([128, NT], dt.float32, tag="po")
                    for k in range(KF):
                        nc.tensor.matmul(
                            pso[:, :w], lhsT=wd_sb[:, k, ts(m, 128)],
                            rhs=h_sbs[s][:, k, :w],
                            start=(k == 0), stop=(k == KF - 1))
                    o_sb = opool.tile([128, NT], dt.float32, tag="o")
                    nc.vector.tensor_copy(o_sb[:, :w], pso[:, :w])
                    nc.sync.dma_start(outT_r[:, m, n0 + s * NT:n0 + s * NT + w],
                                      o_sb[:, :w])

        def body():
            for n0 in range(0, C, NW):
                do_chunk(n0, min(NW, C - n0))

        if hw_loop:
            with tc.For_i(0, hw_loop, 1):
                body()
        else:
            body()
    nc.finalize()
    return nc


def get_nc(C):
    # build_nc_big measured fastest on HW (536 us/core vs 585 us for the
    # 512-chunk baseline, L=64 hw-loop differential timing).
    if C not in _nc_cache:
        _nc_cache[C] = build_nc_big(C)
    return _nc_cache[C]


def build_nc_big(C, hw_loop=0):
    """Variant: 1536-token chunks ([128,1536] 3-bank PSUM tiles).

    Streams 3x512 tokens per weight load (LDW count 1920 -> ~768), evicts
    gate via silu into a chunk-resident SBUF tensor, then multiplies the up
    projection into it in place. PSUM: pp 2x3 + po 2x1 = 8 banks.
    """
    nc = bacc.Bacc("TRN2", target_bir_lowering=False, debug=False, num_devices=E)
    dt = mybir.dt
    NB = 1536
    xT = nc.dram_tensor("xT", [D, C], dt.bfloat16, kind="ExternalInput")
    wgu = nc.dram_tensor("wguT", [D, F2], dt.bfloat16, kind="ExternalInput")
    wd = nc.dram_tensor("wdT", [F, D], dt.bfloat16, kind="ExternalInput")
    outT = nc.dram_tensor("outT", [D, C], dt.float32, kind="ExternalOutput")

    with TileContext(nc) as tc, ExitStack() as ctx:
        wpool = ctx.enter_context(tc.tile_pool(name="weights", bufs=1))
        wgu_sb = wpool.tile([128, KD, F2], dt.bfloat16, tag="wgu")
        nc.sync.dma_start(wgu_sb[:], wgu.rearrange("(k p) f -> p k f", p=128))
        wd_sb = wpool.tile([128, KF, D], dt.bfloat16, tag="wd")
        nc.sync.dma_start(wd_sb[:], wd.rearrange("(k p) f -> p k f", p=128))

        xpool = ctx.enter_context(tc.tile_pool(name="x", bufs=1))
        ghpool = ctx.enter_context(tc.tile_pool(name="gh", bufs=1))
        opool = ctx.enter_context(tc.tile_pool(name="o", bufs=4))
        pp = ctx.enter_context(tc.tile_pool(name="pp", bufs=2, space="PSUM"))
        po = ctx.enter_context(tc.tile_pool(name="po", bufs=2, space="PSUM"))

        xT_r = xT.rearrange("(k p) t -> p k t", p=128)
        outT_r = outT.rearrange("(m p) t -> p m t", p=128)

        def slices(nw):
            return [(s, min(NT, nw - s)) for s in range(0, nw, NT)]

        def do_chunk(n0, nw):
            x_sb = xpool.tile([128, KD, NB], dt.bfloat16, tag="x")
            nc.sync.dma_start(x_sb[:, :, :nw], xT_r[:, :, n0:n0 + nw])
            gh = ghpool.tile([128, KF, NB], dt.bfloat16, tag="gh")
            for phase in (0, 1):  # 0: gate+silu, 1: up+mul-in-place
                for mp in range(KF):
                    ps = pp.tile([128, NB], dt.float32, tag="pp",
                                 name=f"ps_{n0}_{phase}_{mp}")
                    f0 = mp * 128 if phase == 0 else F + mp * 128
                    for k in range(KD):
                        for s, w in slices(nw):
                            nc.tensor.matmul(
                                ps[:, s:s + w],
                                lhsT=wgu_sb[:, k, ds(f0, 128)],
                                rhs=x_sb[:, k, s:s + w],
                                start=(k == 0), stop=(k == KD - 1))
                    if phase == 0:
                        nc.scalar.activation(gh[:, mp, :nw], ps[:, :nw],
                                             mybir.ActivationFunctionType.Silu)
                    else:
                        nc.vector.tensor_mul(gh[:, mp, :nw], gh[:, mp, :nw],
                                             ps[:, :nw])
            for m in range(MD):
                for s, w in slices(nw):
                    pso = po.tile([128, NT], dt.float32, tag="po",
                                  name=f"pso_{n0}_{m}_{s}")
                    for k in range(KF):
                        nc.tensor.matmul(
                            pso[:, :w], lhsT=wd_sb[:, k, ts(m, 128)],
                            rhs=gh[:, k, s:s + w],
                            start=(k == 0), stop=(k == KF - 1))
                    o_sb = opool.tile([128, NT], dt.float32, tag="o",
                                      name=f"o_{n0}_{m}_{s}")
                    nc.vector.tensor_copy(o_sb[:, :w], pso[:, :w])
                    nc.sync.dma_start(outT_r[:, m, n0 + s:n0 + s + w],
                                      o_sb[:, :w])

        def body():
            for n0 in range(0, C, NB):
                do_chunk(n0, min(NB, C - n0))

        if hw_loop:
            with tc.For_i(0, hw_loop, 1):
                body()
        else:
            body()
    nc.finalize()
    return nc


def route(x, expert_indices):
    """Sort tokens by expert; return (order, counts, capacity C)."""
    idx = np.asarray(expert_indices)
    order = np.argsort(idx, kind="stable")
    counts = np.bincount(idx, minlength=E).astype(np.int64)
    C = max(NT, int(-(-counts.max() // 128) * 128))
    return order, counts, C


def make_in_maps(x, expert_indices, gate_up_weight, down_weight):
    order, counts, C = route(x, expert_indices)
    x_sorted = np.asarray(x, dtype=np.float32)[order]
    offs = np.zeros(E + 1, dtype=np.int64)
    np.cumsum(counts, out=offs[1:])
    wguT = np.ascontiguousarray(
        np.transpose(np.asarray(gate_up_weight), (0, 2, 1))).astype(BF16)
    wdT = np.ascontiguousarray(
        np.transpose(np.asarray(down_weight), (0, 2, 1))).astype(BF16)
    in_maps = []
    for e in range(E):
        xe = np.zeros((C, D), dtype=np.float32)
        xe[: counts[e]] = x_sorted[offs[e]: offs[e + 1]]
        in_maps.append({
            "xT": np.ascontiguousarray(xe.T).astype(BF16),
            "wguT": wguT[e],
            "wdT": wdT[e],
        })
    return in_maps, order, counts, C


def assemble_output(results, order, counts):
    T = int(counts.sum())
    out = np.empty((T, D), dtype=np.float32)
    offs = np.zeros(E + 1, dtype=np.int64)
    np.cumsum(counts, out=offs[1:])
    sorted_out = np.empty((T, D), dtype=np.float32)
    for e in range(E):
        sorted_out[offs[e]: offs[e + 1]] = results[e]["outT"].T[: counts[e]]
    out[order] = sorted_out
    return out


def kernel(x, expert_indices, gate_up_weight, down_weight):
    in_maps, order, counts, C = make_in_maps(
        x, expert_indices, gate_up_weight, down_weight)
    nc = get_nc(C)
    res = run_bass_kernel_spmd(nc, in_maps, core_ids=list(range(E)))
    return assemble_output(res.results, order, counts)
